# revision 23
# baseline (speedup 1.0000x reference)
"""Causal GQA self-attention (B=4, T=1024, D=2048, H=16, KVH=4, RoPE) on 8 TRN2 cores.

Sharding: 16 (batch, kv-group) units; core c handles batch c//2 and kv-groups
{2*(c%2), 2*(c%2)+1} (= 8 query heads). Wq/Wkv column-sharded, Wo row-sharded
(Megatron attention TP); each core returns a partial [T, D] output and the host
sums the two partials per batch.

Everything is pre-packed on the host into the exact SBUF layouts the kernel
wants (x is passed transposed), so the device does no data-layout transposes.
All matmuls run as float32r (full PE rate at free-dim >= 256); matmul inputs
are f32r end-to-end (f32r DRAM -> f32r SBUF DMAs satisfy walrus's rounding
rule). RoPE's half-swap crosses partitions, which DVE cannot do, so it uses a
PE permutation matmul:
rope(u) = u * [cosT;cosT] + (SWP @ u) * [-sinT;sinT].
"""

import sys

if "/opt/trn_rl_repo" not in sys.path:
    sys.path.insert(0, "/opt/trn_rl_repo")

from contextlib import ExitStack

import numpy as np

B, T, DIM = 4, 1024, 2048
H, KVH, HD = 16, 4, 128
G = H // KVH
P = 128
KO = DIM // P            # 16 contraction tiles
TT = T // P              # 8 token tiles
HPC = 8                  # heads per core
LG = 2                   # local kv groups per core
QBS = 512                # q block size
NQB = T // QBS           # 2
CBS = 512                # Wo col block size
NCB = DIM // CBS         # 4
SCALE = float(1.0 / np.sqrt(HD))
NCORES = 8

_PROG_CACHE = {}
MM_DTYPE = "f32r"  # or "bf16"


def _build_program():
    import concourse.bacc as bacc
    import concourse.mybir as mybir
    import concourse.tile as tile

    f32 = mybir.dt.float32
    f32r = mybir.dt.float32r if MM_DTYPE == "f32r" else mybir.dt.bfloat16
    EXP = mybir.ActivationFunctionType.Exp

    nc = bacc.Bacc("TRN2", debug=False)

    xt_d = nc.dram_tensor("xt", [P, KO, T], f32r, kind="ExternalInput").ap()
    wq_d = nc.dram_tensor("wq", [P, HPC, KO, P], f32r, kind="ExternalInput").ap()
    wk_d = nc.dram_tensor("wk", [P, KO, LG * HD], f32r, kind="ExternalInput").ap()
    wv_d = nc.dram_tensor("wv", [P, KO, LG * HD], f32r, kind="ExternalInput").ap()
    wo_d = nc.dram_tensor("wo", [P, NCB, HPC, CBS], f32r, kind="ExternalInput").ap()
    cc_d = nc.dram_tensor("cc", [P, T], f32, kind="ExternalInput").ap()
    nss_d = nc.dram_tensor("nss", [P, T], f32, kind="ExternalInput").ap()
    tri_d = nc.dram_tensor("tri", [P, P], f32r, kind="ExternalInput").ap()
    swp_d = nc.dram_tensor("swp", [P, P], f32r, kind="ExternalInput").ap()
    y_d = nc.dram_tensor("y", [T, DIM], f32, kind="ExternalOutput").ap()
    y_r = y_d.rearrange("(to p) c -> p to c", p=P)

    with tile.TileContext(nc) as tc, ExitStack() as ctx:
        const = ctx.enter_context(tc.tile_pool(name="const", bufs=1))
        stream = ctx.enter_context(tc.tile_pool(name="stream", bufs=4))
        xtp = ctx.enter_context(tc.tile_pool(name="xtp", bufs=1))
        big = ctx.enter_context(tc.tile_pool(name="big", bufs=1))
        ptp = ctx.enter_context(tc.tile_pool(name="ptp", bufs=5))
        tmp = ctx.enter_context(tc.tile_pool(name="tmp", bufs=3))
        rec128p = ctx.enter_context(tc.tile_pool(name="rec128p", bufs=1))
        ysbp = ctx.enter_context(tc.tile_pool(name="ysbp", bufs=2))

        mm_ps = ctx.enter_context(tc.tile_pool(name="mm_ps", bufs=2, space="PSUM"))
        s_ps = ctx.enter_context(tc.tile_pool(name="s_ps", bufs=2, space="PSUM"))
        o_ps = ctx.enter_context(tc.tile_pool(name="o_ps", bufs=2, space="PSUM"))
        l_ps = ctx.enter_context(tc.tile_pool(name="l_ps", bufs=2, space="PSUM"))

        ccsb = const.tile([P, T], f32, tag="cc", name="cc")
        nsssb = const.tile([P, T], f32, tag="nss", name="nss")
        trisb = const.tile([P, P], f32r, tag="tri", name="tri")
        swpsb = const.tile([P, P], f32r, tag="swp", name="swp")
        # tri's last column / first row are all-ones: reuse as the ones vector
        ones_col = trisb[:, P - 1 : P]
        ones_row = trisb[0:1, :]

        wvsb = stream.tile([P, KO, LG * HD], f32r, tag="stream", name="wvsb")
        xtsb = xtp.tile([P, KO, T], f32r, tag="xt", name="xt")
        for i in range(KO):
            eng = nc.sync if i % 2 == 0 else nc.scalar
            eng.dma_start(xtsb[:, i : i + 1, :], xt_d[:, i : i + 1, :])
            if i % 4 == 1:
                j = i // 4
                nc.scalar.dma_start(
                    wvsb[:, 4 * j : 4 * j + 4, :], wv_d[:, 4 * j : 4 * j + 4, :]
                )

        nc.sync.dma_start(ccsb[:], cc_d)
        nc.sync.dma_start(nsssb[:], nss_d)
        nc.sync.dma_start(trisb[:], tri_d)
        nc.sync.dma_start(swpsb[:], swp_d)

        qtsb = [big.tile([P, T], f32r, tag=f"qt{h}", name=f"qt{h}") for h in range(HPC)]
        ktsb = big.tile([P, LG, T], f32r, tag="kt", name="kt")
        vsb = big.tile([P, TT, LG * HD], f32r, tag="v", name="v")
        otsb = qtsb  # OT_h overwrites QT_h after the last S^T read of that block

        def rope(src_ps, dst, blk):
            """dst = rope(src_ps) for absolute-t column slice blk.

            usb = copy(src); sw = SWP @ usb;  dst = usb*CC + sw*NSS
            """
            usb = tmp.tile([P, QBS], f32r, tag="tmp", name="usb")
            nc.scalar.copy(usb[:], src_ps[:])
            sw = s_ps.tile([P, QBS], f32, tag="s", name="sw")
            nc.tensor.matmul(sw[:], swpsb[:], usb[:], start=True, stop=True)
            t1 = tmp.tile([P, QBS], f32, tag="tmp", name="t1")
            nc.vector.tensor_mul(t1[:], usb[:], ccsb[:, blk])
            t2 = tmp.tile([P, QBS], f32, tag="tmp", name="t2")
            nc.vector.tensor_mul(t2[:], sw[:], nsssb[:, blk])
            nc.vector.tensor_add(dst, t1[:], t2[:])

        # ---- V projection: V[t, hd] natural layout, both groups at once.
        # kt-outer over tt-pairs so the first matmuls stream with the xt DMA.
        for tp in range(TT // 2):
            vps = [mm_ps.tile([P, QBS], f32, tag="mm", name="vp") for _ in range(2)]
            for kt in range(KO):
                for half in range(2):
                    tt = 2 * tp + half
                    nc.tensor.matmul(
                        vps[half][:, 0 : LG * HD],
                        xtsb[:, kt, tt * P : (tt + 1) * P],
                        wvsb[:, kt, :],
                        start=(kt == 0),
                        stop=(kt == KO - 1),
                    )
            for half in range(2):
                tt = 2 * tp + half
                nc.scalar.copy(vsb[:, tt, :], vps[half][:, 0 : LG * HD])

        # ---- K projection (-> K^T layout [hd, t]) + RoPE ----
        wksb = stream.tile([P, KO, LG * HD], f32r, tag="stream", name="wksb")
        nc.sync.dma_start(wksb[:], wk_d)
        for lg in range(LG):
            for hf in range(NQB):
                blk = slice(hf * QBS, (hf + 1) * QBS)
                kp = mm_ps.tile([P, QBS], f32, tag="mm", name="kp")
                for kt in range(KO):
                    nc.tensor.matmul(
                        kp[:],
                        wksb[:, kt, lg * HD : (lg + 1) * HD],
                        xtsb[:, kt, blk],
                        start=(kt == 0),
                        stop=(kt == KO - 1),
                    )
                rope(kp, ktsb[:, lg, blk], blk)

        # ---- Q projection (-> Q^T layout [hd, t]) + RoPE ----
        for lh in range(HPC):
            wqsb = stream.tile([P, KO, P], f32r, tag="stream", name="wqsb")
            nc.sync.dma_start(wqsb[:], wq_d[:, lh])
            for hf in range(NQB):
                blk = slice(hf * QBS, (hf + 1) * QBS)
                qp = mm_ps.tile([P, QBS], f32, tag="mm", name="qp")
                for kt in range(KO):
                    nc.tensor.matmul(
                        qp[:],
                        wqsb[:, kt, :],
                        xtsb[:, kt, blk],
                        start=(kt == 0),
                        stop=(kt == KO - 1),
                    )
                rope(qp, qtsb[lh][:, blk], blk)

        # ---- attention (q-block outer, head inner) interleaved with Wo ----
        def attn_round(lh, qb):
            lg = lh // 4
            op = o_ps.tile([P, QBS], f32, tag="o", name="op")
            lp = l_ps.tile([1, QBS], f32, tag="l", name="lp")
            items = [(kt, 0, False) for kt in range(4 * qb)]
            items += [(4 * qb + j, P * j, True) for j in range(4)]
            nitems = len(items)
            for idx, (kt, c0, diag) in enumerate(items):
                ncols = QBS - c0
                first = idx == 0
                last = idx == nitems - 1
                spool, stag = ((s_ps, "s") if idx % 2 == 0 else (mm_ps, "mm"))
                sp = spool.tile([P, QBS], f32, tag=stag, name="sp")
                nc.tensor.matmul(
                    sp[:, 0:ncols],
                    ktsb[:, lg, kt * P : (kt + 1) * P],
                    qtsb[lh][:, qb * QBS + c0 : (qb + 1) * QBS],
                    start=True,
                    stop=True,
                )
                pt = ptp.tile([P, QBS], f32r, tag="pt", name="pt")
                nc.scalar.activation(pt[:, c0:QBS], sp[:, 0:ncols], EXP, scale=SCALE)
                if diag:
                    nc.vector.tensor_mul(
                        pt[:, c0 : c0 + P], pt[:, c0 : c0 + P], trisb[:]
                    )
                nc.tensor.matmul(
                    lp[:, c0:QBS], ones_col, pt[:, c0:QBS], start=first, stop=last
                )
                nc.tensor.matmul(
                    op[:, c0:QBS],
                    vsb[:, kt, lg * HD : (lg + 1) * HD],
                    pt[:, c0:QBS],
                    start=first,
                    stop=last,
                )
            # 1/l and its partition-broadcast stay OFF the PE: approx
            # reciprocal on DVE, broadcast on the otherwise-idle GpSimd.
            scratch = tmp.tile([1, QBS], f32, tag="tmp", name="scratch")
            rec = tmp.tile([1, QBS], f32, tag="tmp", name="rec")
            nc.vector.reciprocal_approx_accurate(rec[:], lp[:], scratch[:])
            rec128 = rec128p.tile([P, QBS], f32, tag="rec128", name="rec128")
            nc.gpsimd.partition_broadcast(rec128[:], rec[:])
            nc.vector.tensor_mul(
                otsb[lh][:, qb * QBS : (qb + 1) * QBS], op[:], rec128[:]
            )

        for qb in range(NQB):
            for lh in range(HPC):
                attn_round(lh, qb)

        # ---- output projection: all 4 Wo col-slabs resident; per (tt, lh)
        # one LDW feeds 4 matmuls (one per col block) into 4 psum banks.
        wosbs = []
        for cb in range(NCB):
            w = stream.tile([P, HPC, CBS], f32r, tag="stream", name="wosb")
            nc.sync.dma_start(w[:, 0:4, :], wo_d[:, cb, 0:4, :])
            nc.sync.dma_start(w[:, 4:8, :], wo_d[:, cb, 4:8, :])
            wosbs.append(w)
        yp_pools = [mm_ps, s_ps, o_ps, l_ps]
        for tt in range(TT):
            yps = [
                yp_pools[cb].tile([P, QBS], f32, tag=["mm", "s", "o", "l"][cb],
                                  name="yp")
                for cb in range(NCB)
            ]
            for lh in range(HPC):
                for cb in range(NCB):
                    nc.tensor.matmul(
                        yps[cb][:, 0:CBS],
                        otsb[lh][:, tt * P : (tt + 1) * P],
                        wosbs[cb][:, lh, :],
                        start=(lh == 0),
                        stop=(lh == HPC - 1),
                    )
            for cb in range(NCB):
                ysb = ysbp.tile([P, CBS], f32, tag="ysb", name="ysb")
                nc.scalar.copy(ysb[:], yps[cb][:, 0:CBS])
                nc.sync.dma_start(y_r[:, tt, cb * CBS : (cb + 1) * CBS], ysb[:])

    nc.compile()
    return nc


def _get_program():
    if MM_DTYPE not in _PROG_CACHE:
        _PROG_CACHE[MM_DTYPE] = _build_program()
    return _PROG_CACHE[MM_DTYPE]


def _prep_core(c, x, Wq, Wkv, Wo, cos, sin):
    b = c // 2
    pair = c % 2
    groups = [2 * pair, 2 * pair + 1]
    heads = [g * G + i for g in groups for i in range(G)]

    xT = np.ascontiguousarray(x[b].T)                       # [DIM, T]
    xt_p = np.ascontiguousarray(xT.reshape(KO, P, T).transpose(1, 0, 2))

    wq_cols = np.stack([Wq[:, h * HD : (h + 1) * HD] for h in heads], axis=1)
    wq_p = np.ascontiguousarray(
        wq_cols.reshape(KO, P, HPC, HD).transpose(1, 2, 0, 3)
    )  # [P, lh, kt, c]

    kcols = np.concatenate([Wkv[:, g * HD : (g + 1) * HD] for g in groups], axis=1)
    wk_p = np.ascontiguousarray(kcols.reshape(KO, P, LG * HD).transpose(1, 0, 2))
    vcols = np.concatenate(
        [Wkv[:, KVH * HD + g * HD : KVH * HD + (g + 1) * HD] for g in groups], axis=1
    )
    wv_p = np.ascontiguousarray(vcols.reshape(KO, P, LG * HD).transpose(1, 0, 2))

    worows = np.stack([Wo[h * HD : (h + 1) * HD, :] for h in heads], axis=0)
    wo_p = np.ascontiguousarray(
        worows.reshape(HPC, P, NCB, CBS).transpose(1, 2, 0, 3)
    )  # [P, cb, lh, cc]

    cosT = np.ascontiguousarray(cos.T)                       # [64, T]
    sinT = np.ascontiguousarray(sin.T)
    cc_p = np.ascontiguousarray(np.concatenate([cosT, cosT], axis=0))   # [128, T]
    nss_p = np.ascontiguousarray(np.concatenate([-sinT, sinT], axis=0))
    tri_p = np.triu(np.ones((P, P), dtype=np.float32))
    swp_p = np.roll(np.eye(P, dtype=np.float32), 64, axis=0)  # swp[k,m]=1 iff k=(m+64)%128

    if MM_DTYPE == "f32r":
        mdt = np.float32
    else:
        import ml_dtypes

        mdt = ml_dtypes.bfloat16
    return {
        "xt": xt_p.astype(mdt),
        "wq": wq_p.astype(mdt),
        "wk": wk_p.astype(mdt),
        "wv": wv_p.astype(mdt),
        "wo": wo_p.astype(mdt),
        "cc": cc_p.astype(np.float32, copy=False),
        "nss": nss_p.astype(np.float32, copy=False),
        "tri": tri_p.astype(mdt),
        "swp": swp_p.astype(mdt),
    }


def _run(inputs, trace=False, trace_kwargs=None):
    from concourse import bass_utils

    x = np.asarray(inputs["x"], dtype=np.float32)
    Wq = np.asarray(inputs["Wq"], dtype=np.float32)
    Wkv = np.asarray(inputs["Wkv"], dtype=np.float32)
    Wo = np.asarray(inputs["Wo"], dtype=np.float32)
    cos = np.asarray(inputs["cos"], dtype=np.float32)
    sin = np.asarray(inputs["sin"], dtype=np.float32)

    nc = _get_program()
    in_maps = [_prep_core(c, x, Wq, Wkv, Wo, cos, sin) for c in range(NCORES)]
    kwargs = {}
    if trace:
        kwargs["trace"] = True
        if trace_kwargs:
            kwargs.update(trace_kwargs)
    res = bass_utils.run_bass_kernel_spmd(
        nc, in_maps, core_ids=list(range(NCORES)), **kwargs
    )
    outs = [np.asarray(r["y"], dtype=np.float32) for r in res.results]
    y = np.stack([outs[2 * b] + outs[2 * b + 1] for b in range(B)], axis=0)
    return y, res


def kernel(**inputs):
    y, _ = _run(inputs, trace=False)
    return y
